# revision 39
# baseline (speedup 1.0000x reference)
"""Trainium2 Bass kernel for nn_Head_5128190951491 (Arnold-map attention head).

B=4, T=4096, C=512, D=64. 8 NeuronCores: core c handles batch b=c//2,
sequence-half h=c%2. The host rolls x[b] by -h*2048 rows and ships it
TRANSPOSED ([C, T] contiguous), so every core's query rows are rows
0:2048 of its own input (attention over full T is permutation-invariant
in s) and the device needs no transpose pass for x^T.

Per-core device program:
  phase A: DMA x^T in chunks (staged fp32, rounded to f32r by DVE/ACT
           copies); f32r projections q^T,k^T,v^T (q first, so the Arnold
           map + qT dup overlap the k/v projections); Arnold map on q,k
           (4-op DVE chain + ACT Sin with the range reduction folded into
           the Sin's affine bias); v^T -> PE transpose (batched 8/psum
           bank) -> v_aug [s,65] bf16 (col 64 = ones for softmax sums).
  phase B: per q-block(512): S^T = k^T.T @ q^T (K=64 matmuls packed on
           partition halves), exp via ACT (scale=1/8, bf16 out) over
           2048/1024-wide PSUM chunks, PV: o_aug^T[65,512] += v_aug.T @
           expS^T accumulated over s. Tail: transpose o_aug^T, divide by
           row sums, DMA out.

The whole program can be wrapped in a For_i rep loop (build(reps=R)) so
one dispatch runs the computation R times back-to-back on device; the
timing path differences two rep counts to isolate per-execution hardware
time from the ~1.6 ms axon RPC overhead per dispatch.
"""

import sys
import types

sys.path.insert(0, "/opt/trn_rl_repo")

import numpy as np

# antenv.axon_hooks is absent in this container; stub it so
# run_bass_kernel_spmd's axon path degrades gracefully.
try:
    import antenv.axon_hooks  # noqa: F401
except ImportError:
    import antenv

    _m = types.ModuleType("antenv.axon_hooks")
    _m.get_axon_ntff_profile_hook = lambda: None
    sys.modules["antenv.axon_hooks"] = _m
    antenv.axon_hooks = _m

import concourse.bass as bass
import concourse.mybir as mybir
import concourse.tile as tile
from concourse import bacc
from concourse.bass import ts
from concourse.bass_utils import run_bass_kernel_spmd
from concourse.masks import make_identity

OMEGA = 0.618
B, T, C, D = 4, 4096, 512, 64
NCORES = 8
TH = T // 2  # 2048 query rows per core
FP32 = mybir.dt.float32
F32R = mybir.dt.float32r
BF16 = mybir.dt.bfloat16
I32 = mybir.dt.int32
AF = mybir.ActivationFunctionType
ALU = mybir.AluOpType

_CACHE = {}


def _arnold_chain(nc, pool, src_ap, dst_ap, c1, p, n, tagp=""):
    """dst = mod(src + OMEGA - c1*sin(2pi*src), 1.0). src fp32 [p,n] (SBUF or
    PSUM), dst bf16 [p,n]. Elementwise on DVE; the Sin on ACT.

    Fast path (c1 <= 0.38): fold the range reduction into one rounded
    subtract. With i2 = rint(src + 0.5), g = (src + OMEGA) - i2 lies in
    [OMEGA-1, OMEGA); s = Sin(2pi*g + 2pi*(0.5-OMEGA)) = sin(2pi*r)
    = -sin(2pi*src) evaluates exactly inside the spline's valid [-pi, pi)
    domain, the sign folds into +c1, and w = g + c1*s is in (-1, 1), so a
    single negative-wrap fix gives mod(.., 1).
    """
    two_pi = float(np.float32(2.0 * np.pi))
    pi = float(np.float32(np.pi))
    if c1 <= 0.38:
        i2 = pool.tile([p, n], I32, tag=tagp + "arn_i")
        nc.vector.tensor_scalar(i2[:], src_ap, 0.5, None, op0=ALU.add)
        g = pool.tile([p, n], FP32, tag=tagp + "arn_a")
        nc.vector.scalar_tensor_tensor(
            g[:], src_ap, OMEGA, i2[:], op0=ALU.add, op1=ALU.subtract
        )
        s = pool.tile([p, n], FP32, tag=tagp + "arn_s")
        bco = pool.tile([p, 1], FP32, tag=tagp + "arn_pi")
        nc.vector.memset(bco[:], float(np.float32(two_pi * (0.5 - OMEGA))))
        nc.scalar.activation(s[:], g[:], AF.Sin, scale=two_pi, bias=bco[:])
        w = pool.tile([p, n], FP32, tag=tagp + "arn_b")
        nc.vector.scalar_tensor_tensor(
            w[:], s[:], c1, g[:], op0=ALU.mult, op1=ALU.add
        )
        nc.vector.scalar_tensor_tensor(
            dst_ap, w[:], 0.0, w[:], op0=ALU.is_lt, op1=ALU.add
        )
        return
    # general path: full frac() twice (any c1)
    i2 = pool.tile([p, n], I32, tag=tagp + "arn_i")
    nc.vector.tensor_scalar(i2[:], src_ap, 0.5, None, op0=ALU.add)
    f2 = pool.tile([p, n], FP32, tag=tagp + "arn_a")
    nc.vector.scalar_tensor_tensor(
        f2[:], src_ap, 0.5, i2[:], op0=ALU.add, op1=ALU.subtract
    )
    q0 = pool.tile([p, n], FP32, tag=tagp + "arn_b")
    nc.vector.scalar_tensor_tensor(
        q0[:], f2[:], 0.0, f2[:], op0=ALU.is_lt, op1=ALU.add
    )
    s = pool.tile([p, n], FP32, tag=tagp + "arn_s")
    mpi = pool.tile([p, 1], FP32, tag=tagp + "arn_pi")
    nc.vector.memset(mpi[:], -pi)
    nc.scalar.activation(s[:], q0[:], AF.Sin, scale=two_pi, bias=mpi[:])
    u = pool.tile([p, n], FP32, tag=tagp + "arn_a")
    nc.vector.scalar_tensor_tensor(
        u[:], s[:], -c1, src_ap, op0=ALU.mult, op1=ALU.add
    )
    i = pool.tile([p, n], I32, tag=tagp + "arn_i")
    nc.vector.tensor_scalar(i[:], u[:], OMEGA, None, op0=ALU.add)
    f = pool.tile([p, n], FP32, tag=tagp + "arn_b")
    nc.vector.scalar_tensor_tensor(
        f[:], u[:], OMEGA, i[:], op0=ALU.add, op1=ALU.subtract
    )
    nc.vector.scalar_tensor_tensor(
        dst_ap, f[:], 0.0, f[:], op0=ALU.is_lt, op1=ALU.add
    )


def build(c1: float, reps: int = 1):
    nc = bacc.Bacc("TRN2", target_bir_lowering=False, debug=False,
                   num_devices=NCORES)
    xr = nc.dram_tensor("xr", [C, T], FP32, kind="ExternalInput")
    wqt = nc.dram_tensor("wqt", [C, D], FP32, kind="ExternalInput")
    wkt = nc.dram_tensor("wkt", [C, D], FP32, kind="ExternalInput")
    wvt = nc.dram_tensor("wvt", [C, D], FP32, kind="ExternalInput")
    out = nc.dram_tensor("out", [TH, D], FP32, kind="ExternalOutput")

    NCT = C // 128      # 4 c-tiles
    NTB = T // 512      # 8 t-blocks
    NQB = TH // 512     # 4 q t-blocks
    NST = T // 128      # 32 s-tiles

    with tile.TileContext(nc) as tc:
        # Device-side rep loop (static bound; dynamic bounds fail under the
        # axon execute path). reps=1 (no loop) for the correctness path;
        # reps=R for timing, where one dispatch re-runs the whole program R
        # times so per-exec device time can be isolated from RPC overhead.
        loop_cm = tc.For_i(0, reps, name="reploop") if reps > 1 else None
        if loop_cm is not None:
            loop_cm.__enter__()
        with tc.tile_pool(name="big", bufs=1) as big:
          with (
            tc.tile_pool(name="xstage", bufs=2) as xstage,
            tc.tile_pool(name="tposA", bufs=2, space="PSUM") as tposA,
            tc.tile_pool(name="projp", bufs=2, space="PSUM") as projp,
            tc.tile_pool(name="arn", bufs=1) as arn,
          ):
            ident = big.tile([128, 128], BF16)
            make_identity(nc, ident[:])
            identf = big.tile([128, 128], FP32)
            make_identity(nc, identf[:])

            # x^T in f32r: [4 c-tiles][128, T]
            xT = big.tile([128, NCT, T], F32R)
            # weights W^T: [C=4*128, 64] f32r  (DMA fp32 then round)
            w_sb = big.tile([128, NCT, 3 * D], FP32)
            for wi, w in enumerate((wqt, wkt, wvt)):
                nc.sync.dma_start(
                    w_sb[:, :, ts(wi, D)],
                    w.ap().rearrange("(ct p) d -> p ct d", p=128),
                )
            w_r = big.tile([128, NCT, 3 * D], F32R)
            nc.vector.tensor_copy(w_r[:], w_sb[:])

            # ---- phase A: x^T arrives pre-transposed from the host ----
            # Bulk load on qSP in 8 chunks; the latency-critical small
            # repack/dup DMAs get their own queue (qAct) so they are not
            # FIFO-blocked behind the full load. f32r matmul operands must
            # be produced by a rounding instruction, so each chunk stages
            # fp32 and a DVE/ACT copy rounds it into xT (both engines are
            # otherwise idle during the load).
            NXC = 8
            XCW = T // NXC
            for xc in range(NXC):
                xs = xstage.tile([128, NCT, XCW], FP32)
                deng = nc.sync if xc % 2 == 0 else nc.scalar
                deng.dma_start(
                    xs[:],
                    xr.ap()[:, ts(xc, XCW)].rearrange(
                        "(ct p) t -> p ct t", p=128),
                )
                if xc % 2 == 0:
                    nc.vector.tensor_copy(xT[:, :, ts(xc, XCW)], xs[:])
                else:
                    nc.scalar.copy(xT[:, :, ts(xc, XCW)], xs[:])

            # ---- projections ([64, 512] PSUM tiles; q first so arnold
            # q + the qT dup can overlap the k/v projections) ----
            # kT packed layout: rows 0-63 = s in [0,2048), rows 64-127 =
            # s in [2048,4096), columns = s % 2048. QK pairs (sj, sj+16).
            kT = big.tile([128, TH], BF16)
            qT = big.tile([128, TH], BF16)        # q duplicated both halves
            vT = big.tile([64, T], BF16)          # v^T (plain)
            q32p = big.tile([128, 1024], FP32)
            k32a = big.tile([128, 1024], FP32)
            k32b = big.tile([128, 1024], FP32)
            qb = big.tile([128, 1024], BF16)
            c1f = float(np.float32(c1))
            kq32 = big.tile([64, T + TH], FP32)  # k^T | q^T pre-arnold

            def proj(wi, tb, col, dst_copy):
                pk = projp.tile([64, 512], FP32, tag="proj")
                for ct in range(NCT):
                    nc.tensor.matmul(
                        pk[:],
                        w_r[:, ct, ts(wi, D)].bitcast(F32R),
                        xT[:, ct, ts(tb, 512)].bitcast(F32R),
                        start=(ct == 0),
                        stop=(ct == NCT - 1),
                    )
                dst_copy(kq32[:, ts(col, 512)], pk[:])

            for tb in range(NQB):   # q -> kq32 cols [4096, 6144)
                proj(0, tb, NTB + tb, nc.vector.tensor_copy)
            # pack q at 128-partition width, arnold, dup for the QK rhs
            nc.scalar.dma_start(q32p[0:64, :], kq32[:, 4096:5120])
            nc.scalar.dma_start(q32p[64:128, :], kq32[:, 5120:6144])
            _arnold_chain(nc, arn, q32p[:], qb[:], c1f, 128, 1024)
            # qb rows 0-63 = q t in [0,1024), rows 64-127 = t in [1024,2048)
            nc.scalar.dma_start(qT[0:64, 0:1024], qb[0:64, :])
            nc.scalar.dma_start(qT[0:64, 1024:2048], qb[64:128, :])
            nc.scalar.dma_start(qT[64:128, 0:1024], qb[0:64, :])
            nc.scalar.dma_start(qT[64:128, 1024:2048], qb[64:128, :])

            # k chunk a: s [0,1024) | [2048,3072), then chunk b
            for tb in (0, 1, 4, 5):
                proj(1, tb, tb, nc.vector.tensor_copy)
            nc.scalar.dma_start(k32a[0:64, :], kq32[:, 0:1024])
            nc.scalar.dma_start(k32a[64:128, :], kq32[:, 2048:3072])
            _arnold_chain(nc, arn, k32a[:], kT[:, 0:1024], c1f, 128, 1024)
            for tb in (2, 3, 6, 7):
                proj(1, tb, tb, nc.vector.tensor_copy)
            nc.scalar.dma_start(k32b[0:64, :], kq32[:, 1024:2048])
            nc.scalar.dma_start(k32b[64:128, :], kq32[:, 3072:4096])

            # v projections before arnold k2: vT copies ride on ACT between
            # the k1 and k2 sins, so ACT is clear for exps when QK starts.
            v_aug = big.tile([128, NST, 72], BF16)
            nc.gpsimd.memset(v_aug[:], 1.0)
            for tb in range(NTB):
                pv = projp.tile([64, 512], FP32, tag="projv")
                for ct in range(NCT):
                    nc.tensor.matmul(
                        pv[:],
                        w_r[:, ct, ts(2, D)].bitcast(F32R),
                        xT[:, ct, ts(tb, 512)].bitcast(F32R),
                        start=(ct == 0),
                        stop=(ct == NCT - 1),
                    )
                nc.scalar.copy(vT[:, ts(tb, 512)], pv[:])
            _arnold_chain(nc, arn, k32b[:], kT[:, 1024:2048], c1f, 128, 1024,
                          tagp="b_")
            # v_aug [128 s, 32 si, 65] with ones col: 8 transposes batch
            # into one PSUM bank + one wide DVE copy (a per-tile
            # transpose->copy ping-pong serializes at ~0.5us a pair).
            for sb in range(NST // 8):
                pt = tposA.tile([128, 512], BF16, tag="vtp")
                for sj in range(8):
                    nc.tensor.transpose(
                        pt[:, ts(sj, 64)],
                        vT[:, ts(sb * 8 + sj, 128)], ident[:64, :64])
                nc.vector.tensor_copy(
                    v_aug[:, sb * 8:(sb + 1) * 8, 0:64],
                    pt[:].rearrange("p (i d) -> p i d", d=64))

          with (
            tc.tile_pool(name="sps", bufs=2, space="PSUM") as sps,
            tc.tile_pool(name="ops", bufs=2, space="PSUM") as ops_p,
            tc.tile_pool(name="tps", bufs=2, space="PSUM") as tps,
            tc.tile_pool(name="expp", bufs=4) as expp,
            tc.tile_pool(name="outp", bufs=3) as outp,
          ):
            # ---- phase B ----
            for tb in range(NQB):
                po = ops_p.tile([65, 512], FP32, tag="po")
                for sj in range(NST // 2):
                    pS = sps.tile([128, 1024], FP32, tag="pS")
                    for k2 in range(2):
                        r0 = 64 * k2   # kT packed: si>=16 on rows 64-127
                        nc.tensor.matmul(
                            pS[:, ts(k2, 512)],
                            kT[r0:r0 + 64, ts(sj, 128)],
                            qT[r0:r0 + 64, ts(tb, 512)],
                            start=True,
                            stop=True,
                            tile_position=(r0, 0),
                        )
                    eS = expp.tile([128, 1024], BF16, tag="eS")
                    nc.scalar.activation(eS[:], pS[:], AF.Exp, scale=0.125)
                    for k2 in range(2):
                        si = sj + 16 * k2
                        nc.tensor.matmul(
                            po[:],
                            v_aug[:, si, 0:65],
                            eS[:, ts(k2, 512)],
                            start=(sj == 0 and k2 == 0),
                            stop=(sj == NST // 2 - 1 and k2 == 1),
                        )
                # tail: transpose 4x[65,128] -> [128,65], normalize, out
                o_sb = outp.tile([65, 512], FP32, tag="osb")
                nc.vector.tensor_copy(o_sb[:], po[:])
                for q4 in range(4):
                    pt = tps.tile([128, 65], FP32, tag="pt")
                    nc.tensor.transpose(
                        pt[:], o_sb[:, ts(q4, 128)], identf[:65, :65]
                    )
                    rz = outp.tile([128, 1], FP32, tag="rz")
                    nc.vector.reciprocal(rz[:], pt[:, 64:65])
                    ot = outp.tile([128, D], FP32, tag="ot")
                    nc.vector.tensor_scalar(
                        ot[:], pt[:, 0:64], rz[:], None, op0=ALU.mult
                    )
                    nc.sync.dma_start(
                        out.ap()[tb * 512 + q4 * 128:tb * 512 + (q4 + 1) * 128, :],
                        ot[:],
                    )

        if loop_cm is not None:
            loop_cm.__exit__(None, None, None)

    nc.compile()
    return nc


def _make_in_maps(x, Wq, Wk, Wv):
    wqt = np.ascontiguousarray(np.asarray(Wq, np.float32).T)
    wkt = np.ascontiguousarray(np.asarray(Wk, np.float32).T)
    wvt = np.ascontiguousarray(np.asarray(Wv, np.float32).T)
    in_maps = []
    xt_cache = {}
    for c in range(NCORES):
        b, h = c // 2, c % 2
        if (b, h) not in xt_cache:
            xb = x[b] if h == 0 else np.roll(x[b], -TH, axis=0)
            # host supplies x^T ([C, T] contiguous): layout prep, same bits
            xt_cache[(b, h)] = np.ascontiguousarray(xb.T)
        in_maps.append({
            "xr": xt_cache[(b, h)],
            "wqt": wqt, "wkt": wkt, "wvt": wvt,
        })
    return in_maps


def _make_sharded_runner(nc, in_maps):
    """jit-wrap nc's NEFF for 8-core SPMD execution with device-resident
    inputs. Returns run_k(k) that walls k pipelined dispatches."""
    import time

    import jax
    from jax.sharding import Mesh, NamedSharding, PartitionSpec
    from jax.experimental.shard_map import shard_map

    from concourse import bass2jax, mybir as mb

    bass2jax.install_neuronx_cc_hook()
    partition_name = (nc.partition_id_tensor.name
                      if nc.partition_id_tensor else None)
    in_names, out_names, out_avals, zero_outs = [], [], [], []
    for alloc in nc.m.functions[0].allocations:
        if not isinstance(alloc, mb.MemoryLocationSet):
            continue
        name = alloc.memorylocations[0].name
        if alloc.kind == "ExternalInput":
            if name != partition_name:
                in_names.append(name)
        elif alloc.kind == "ExternalOutput":
            dt = mb.dt.np(alloc.dtype)
            out_names.append(name)
            out_avals.append(jax.core.ShapedArray(tuple(alloc.tensor_shape), dt))
            zero_outs.append(np.zeros(tuple(alloc.tensor_shape), dt))
    n_params = len(in_names)
    all_in = in_names + out_names
    if partition_name is not None:
        all_in.append(partition_name)

    def _body(*args):
        operands = list(args)
        if partition_name is not None:
            operands.append(bass2jax.partition_id_tensor())
        return tuple(bass2jax._bass_exec_p.bind(
            *operands,
            out_avals=tuple(out_avals),
            in_names=tuple(all_in),
            out_names=tuple(out_names),
            lowering_input_output_aliases=(),
            sim_require_finite=True,
            sim_require_nnan=True,
            nc=nc,
        ))

    devices = jax.devices()[:NCORES]
    mesh = Mesh(np.asarray(devices), ("core",))
    nin = n_params + len(zero_outs)
    sharded = jax.jit(
        shard_map(_body, mesh=mesh, in_specs=(PartitionSpec("core"),) * nin,
                  out_specs=(PartitionSpec("core"),) * len(out_names),
                  check_rep=False),
        keep_unused=True,
    )
    sh = NamedSharding(mesh, PartitionSpec("core"))
    per_core = [[np.asarray(m[nm]) for nm in in_names] for m in in_maps]
    concat = [np.concatenate([per_core[c][i] for c in range(NCORES)], axis=0)
              for i in range(n_params)]
    dev_in = [jax.device_put(a, sh) for a in concat]
    dev_z = [jax.device_put(
        np.zeros((NCORES * z.shape[0], *z.shape[1:]), z.dtype), sh)
        for z in zero_outs]

    def run_k(k):
        t0 = time.perf_counter()
        outs = [sharded(*dev_in, *dev_z) for _ in range(k)]
        jax.block_until_ready(outs)
        return time.perf_counter() - t0

    return run_k


def time_device_exec(inputs, iters=4, r_lo=65, r_hi=129, n_pipe=12):
    """Measure per-execution device time of the kernel NEFF.

    The axon relay adds ~1.6 ms RPC overhead per dispatch, which buries
    the actual on-device execution (~1e2 us). We build the same program
    wrapped in a device-side For_i rep loop at two trip counts; one
    dispatch runs the whole computation r times back-to-back on device.
    Differencing two large rep counts over k pipelined dispatches cancels
    the (noisy) fixed RPC/latency term:

        t_exec = (wall[r_hi] - wall[r_lo]) / (k * (r_hi - r_lo))

    All inputs are device-resident; nothing transfers inside the timed
    region. Returns ns per kernel execution (incl. the per-iteration loop
    barrier and ACT table reloads, so slightly conservative).
    """
    x = np.asarray(inputs["x"], np.float32)
    c1 = float(np.float32(np.abs(np.float32(np.asarray(inputs["K"]).reshape(-1)[0])))
               / np.float32(2.0 * np.pi))
    in_maps = _make_in_maps(x, inputs["Wq"], inputs["Wk"], inputs["Wv"])

    runners = {}
    for r in (r_lo, r_hi):
        key = (round(c1 * 1e9), r)
        if key not in _CACHE:
            _CACHE[key] = build(c1, reps=r)
        runners[r] = _make_sharded_runner(_CACHE[key], in_maps)

    # warmup (compiles jits, warms relay path + PE clock)
    runners[r_lo](2)
    runners[r_hi](2)

    best = float("inf")
    for _ in range(iters):
        wl = runners[r_lo](n_pipe)
        wh = runners[r_hi](n_pipe)
        t_exec = (wh - wl) / (n_pipe * (r_hi - r_lo))
        best = min(best, t_exec)
        print("  wall reps=%d x%d: %.1f ms | reps=%d x%d: %.1f ms -> %.1f us/exec"
              % (r_lo, n_pipe, wl * 1e3, r_hi, n_pipe, wh * 1e3, t_exec * 1e6))
    return int(best * 1e9)


def kernel(x, Wq, Wk, Wv, K):
    x = np.asarray(x, dtype=np.float32)
    c1 = float(np.float32(np.abs(np.float32(K.reshape(-1)[0])))
               / np.float32(2.0 * np.pi))
    key = (round(c1 * 1e9), 1)
    if key not in _CACHE:
        _CACHE[key] = build(c1, reps=1)
    nc = _CACHE[key]

    in_maps = _make_in_maps(x, Wq, Wk, Wv)
    res = run_bass_kernel_spmd(nc, in_maps, core_ids=list(range(NCORES)))
    outp = np.empty((B, T, D), dtype=np.float32)
    for c in range(NCORES):
        b, h = c // 2, c % 2
        outp[b, h * TH:(h + 1) * TH, :] = res.results[c]["out"]
    return outp
